# revision 12
# baseline (speedup 1.0000x reference)
"""Trainium2 Bass kernel for nn_MultiHeadModulator (8-core SPMD).

Math reformulation (exact): with a single query q = Wq@z_curr+bq,
  - dot scores:  score[l,h] = z[l]·A[:,h] + c[h],   A[:,h] = Wk[hb,:]^T @ q[hb]
  - rel scores fold into a per-(l,h) additive bias known on the host
  - value sum:   sum_l e[l,h]*v[l] = Wv @ (sum_l e[l,h]*z[l]) + (sum_l e[l,h])*bv
so the device only computes, per L-shard:
  sc_T[l,h] = z^T A + cb  (PE; zt stationary, A moving -> scores land
                           TRANSPOSED [128 l-part, 4, 8] so heads sit in the
                           free dim and l on partitions)
  e8 = exp(SCALE*sc_T)    (ACT on 128 partitions, fp8 out = the U weights)
  U[h,:] += e8^T z        (PE, fp8 DoubleRow, zn moving)
  S[h]   += e8^T 1        (PE, ones column, same loaded weights)
and the host applies Wv/Wo and the softmax normalization to the tiny [8,512]
all-core sums.  Softmax runs without max-subtraction: scores are O(1) by
construction (validated |score| < 3).

The transposed-score dataflow keeps the per-block cross-engine chain at two
sem hops (PE scores -> ACT exp -> PE U); the PE transposes of e^T, the DVE
fp8 cast, and the per-block accumulator reads of the previous design all
disappear.  cb enters the score PSUM group as a rank-1 bf16 matmul
(ones[1,128]^T @ cbrow); the rel-bias correction is a DVE add on the score
PSUM (block 0 only in the common curr_pos regime).

Sharding: z_past split into 8 contiguous shards of 8192 rows, one per core.
The host ships each shard twice (feature-major zt for scores, row-major zn
for U) in fp8, pre-packed for DoubleRow access: the dual layout costs 2x HBM
but beats any on-chip transpose of z (PE transpose of z would cost more PE
time than the whole matmul pipeline; DVE/XBAR transposes are slower still).

Scheduling notes (hard-won):
  - z streams as 16 half-super chunks (512 KB) zt/zn interleaved on the sync
    HWDGE ring, dispatched before the compute loop in consumption order.
    Coarse 1 MB chunks make the PE wait on super boundaries, which de-ramps
    the PE clock (p-state) and doubles matmul issue time for ~3 us after.
  - consts ride the scalar ring; gpsimd/SWDGE first-byte is ~10 us.
  - per-engine program order is a hard serialization (tick-counter sems):
    emit U(b-1) after scores(b) so the PE covers the ACT exp latency; never
    emit PE work that waits on engines more than ~1 block ahead.
  - weight-side DoubleRow LDWEIGHTS requires the pair-dim step to be a
    multiple of 16 elements (zt's d-pair stride is 512: fine).
  - PSUM budget (8 banks): 3x sc_T + 2x U (even/odd blocks, so the even
    half's flush copy hides under block 15) + 1x S.
"""

import numpy as np
import ml_dtypes

import concourse.bass as bass  # noqa: F401  (engine namespaces live on the nc)
import concourse.mybir as mybir
import concourse.tile as tile
from concourse import bacc
from concourse.bass_utils import run_bass_kernel_spmd

HEADS = 8
REL_MAX = 64
DIM = 256
D2 = 512                      # flattened real feature dim
HD = DIM // HEADS             # 32 complex => 64 reals per head block
L_TOTAL = 65536
N_CORES = 8
L_SHARD = L_TOTAL // N_CORES  # 8192
N_BLOCKS = L_SHARD // 512     # 16 blocks of 512 rows
BLK_PER_SUPER = 4             # host packing granularity (1 MB supers)
N_SUPER = N_BLOCKS // BLK_PER_SUPER
SCALE = 1.0 / np.sqrt(HD)

FP8 = ml_dtypes.float8_e4m3   # == mybir.dt.float8e4 (trainium E4M3, max 240)
BF16 = ml_dtypes.bfloat16

TRACE = False                 # test.py can flip this for profiling runs
TRACE_KW = {}

_cached = {}


def _build_program(full_fac: bool):
    nc = bacc.Bacc(
        "TRN2", target_bir_lowering=False, debug=False, num_devices=N_CORES
    )
    DR = mybir.MatmulPerfMode.DoubleRow
    f8 = mybir.dt.float8e4
    bf = mybir.dt.bfloat16
    f32 = mybir.dt.float32

    ZT = nc.dram_tensor(
        "zt", [N_SUPER, 128, BLK_PER_SUPER, 2, 2, 512], f8, kind="ExternalInput"
    )
    ZN = nc.dram_tensor(
        "zn", [N_SUPER, 128, BLK_PER_SUPER, 2, 2, 512], f8, kind="ExternalInput"
    )
    AT = nc.dram_tensor("a_dr", [128, 2, 2, 16], f8, kind="ExternalInput")
    CBR = nc.dram_tensor("cbr", [1, 4, 16], bf, kind="ExternalInput")
    N_LNF = N_BLOCKS if full_fac else 1
    LNF = nc.dram_tensor("lnf", [128, N_LNF, 4, 8], f32, kind="ExternalInput")
    # output: cols 0..511 = U (even blocks), 512..1023 = U (odd blocks),
    # cols 1024..1039 = S (col 1024 live); host sums the two U halves
    OUT = nc.dram_tensor("out", [8, 1040], f32, kind="ExternalOutput")

    with tile.TileContext(nc) as tc:
        with (
            tc.tile_pool(name="zt", bufs=N_BLOCKS // 2) as zt_pool,
            tc.tile_pool(name="zn", bufs=N_BLOCKS // 2) as zn_pool,
            tc.tile_pool(name="consts", bufs=1) as const_pool,
            tc.tile_pool(name="e8", bufs=6) as e8_pool,
            tc.tile_pool(name="outs", bufs=1) as out_pool,
            tc.tile_pool(name="ps_sc", bufs=3, space="PSUM") as sc_pool,
            tc.tile_pool(name="ps_acc", bufs=1, space="PSUM") as acc_pool,
        ):
            # consts ride the scalar ring (tiny; the sync ring is fully
            # booked streaming z in consumption order)
            a_sb = const_pool.tile([128, 2, 2, 16], f8)
            nc.scalar.dma_start(a_sb[:], AT[:])
            cbr_sb = const_pool.tile([1, 4, 16], bf)
            nc.scalar.dma_start(cbr_sb[:], CBR[:])
            lnf_sb = const_pool.tile([128, N_LNF, 4, 8], f32)
            nc.scalar.dma_start(lnf_sb[:], LNF[:])
            # ones constants are generated on idle engines, not shipped
            ones_bf = const_pool.tile([1, 128], bf)
            nc.vector.memset(ones_bf[:], 1.0)
            ones8 = const_pool.tile([128, 2, 16], f8)
            nc.vector.memset(ones8[:], 1.0)

            # z streams as 16 half-super chunks (2 blocks each), zt/zn
            # interleaved, so the PE never waits on a coarse 1 MB completion
            # semaphore at super boundaries (those stalls de-ramp the PE
            # clock)
            N_HALF = N_BLOCKS // 2
            zt_tiles = [None] * N_HALF
            zn_tiles = [None] * N_HALF
            for h in range(N_HALF):
                sup, hb = divmod(h, 2)
                zt_h = zt_pool.tile([128, 2, 2, 2, 512], f8, tag="zt_h")
                nc.sync.dma_start(zt_h[:], ZT[sup][:, 2 * hb : 2 * hb + 2])
                zt_tiles[h] = zt_h
                zn_h = zn_pool.tile([128, 2, 2, 2, 512], f8, tag="zn_h")
                nc.sync.dma_start(zn_h[:], ZN[sup][:, 2 * hb : 2 * hb + 2])
                zn_tiles[h] = zn_h

            # two U accumulators (even/odd blocks) so the even half's
            # PSUM->SBUF copy overlaps the last block's compute
            u_even = acc_pool.tile([8, 512], f32, tag="u0")
            u_odd = acc_pool.tile([8, 512], f32, tag="u1")
            u_ps = [u_even, u_odd]
            s_ps = acc_pool.tile([8, 16], f32, tag="s")
            out_sb = out_pool.tile([8, 1040], f32)

            e8s = [None] * N_BLOCKS

            def emit_scores(b):
                zt_t = zt_tiles[b // 2][:, b % 2]  # [128, 2(c), 2(d), 512(l)]
                sc = sc_pool.tile([128, 4, 16], f32, tag="sc")
                # rank-1 cb seed opens the accumulation group (writes cb to
                # every l row); the 8 DoubleRow chunk matmuls accumulate the
                # dot products on top, one stop per l-chunk
                nc.tensor.matmul(
                    sc[:], ones_bf[:], cbr_sb[:], start=True, stop=False,
                    skip_group_check=True,
                )
                for q in range(4):
                    for cpair in range(2):
                        nc.tensor.matmul(
                            sc[:, q],
                            zt_t[:, cpair, :, 128 * q : 128 * (q + 1)],
                            a_sb[:, cpair],
                            start=False,
                            stop=(cpair == 1),
                            perf_mode=DR,
                            skip_group_check=True,
                        )
                if full_fac or b == 0:
                    # additive rel-bias correction on the raw scores (exact;
                    # the common curr_pos regime only needs it for block 0)
                    nc.vector.tensor_add(
                        sc[:, :, 0:8], sc[:, :, 0:8],
                        lnf_sb[:, b if full_fac else 0],
                    )
                return sc

            def emit_exp(b, sc):
                e8 = e8_pool.tile([128, 4, 16], f8, tag="e8")
                nc.scalar.activation(
                    e8[:, :, 0:8],
                    sc[:, :, 0:8],
                    mybir.ActivationFunctionType.Exp,
                    scale=float(SCALE),
                )
                e8s[b] = e8

            def emit_u(b):
                zn_t = zn_tiles[b // 2][:, b % 2]  # [128, 2(s), 2(d), 512(f)]
                e8 = e8s[b]
                par = b & 1
                for s in range(2):
                    nc.tensor.matmul(
                        u_ps[par][:],
                        e8[:, 2 * s : 2 * s + 2, 0:8],
                        zn_t[:, s],
                        start=(b == par and s == 0),
                        stop=(b >= N_BLOCKS - 2 and s == 1),
                        perf_mode=DR,
                    )
                    # S rides the already-loaded weights: one extra column
                    nc.tensor.matmul(
                        s_ps[:, 0:1],
                        e8[:, 2 * s : 2 * s + 2, 0:8],
                        ones8[:, :, 0:1],
                        start=(b == 0 and s == 0),
                        stop=(b == N_BLOCKS - 1 and s == 1),
                        perf_mode=DR,
                    )

            for b in range(N_BLOCKS):
                sc = emit_scores(b)
                if b > 0:
                    emit_u(b - 1)
                emit_exp(b, sc)
                if b == N_BLOCKS - 1:
                    # even-half flush overlaps block 15's compute (DVE copy)
                    nc.vector.tensor_copy(out_sb[:, 0:512], u_ps[0][:])
                    nc.sync.dma_start(OUT[:, 0:512], out_sb[:, 0:512])
            emit_u(N_BLOCKS - 1)

            # final (odd) U + S copies on ACT, idle after exp(15)
            nc.scalar.copy(out_sb[:, 512:1024], u_ps[1][:])
            nc.scalar.copy(out_sb[:, 1024:1040], s_ps[:])
            nc.sync.dma_start(OUT[:, 512:1040], out_sb[:, 512:1040])

    nc.compile()
    return nc


def _get_program(full_fac: bool):
    if full_fac not in _cached:
        _cached[full_fac] = _build_program(full_fac)
    return _cached[full_fac]


def kernel(curr_pos, z_curr, z_past, Wq, bq, Wk, bk, Wv, bv, Wo, bo, rel_bias):
    curr_pos = int(np.asarray(curr_pos))
    z_curr = np.asarray(z_curr, dtype=np.float32)
    z_past = np.asarray(z_past, dtype=np.float32)
    Wq = np.asarray(Wq, dtype=np.float32)
    bq = np.asarray(bq, dtype=np.float32)
    Wk = np.asarray(Wk, dtype=np.float32)
    bk = np.asarray(bk, dtype=np.float32)
    Wv = np.asarray(Wv, dtype=np.float32)
    bv = np.asarray(bv, dtype=np.float32)
    Wo = np.asarray(Wo, dtype=np.float32)
    bo = np.asarray(bo, dtype=np.float32)
    rel_bias = np.asarray(rel_bias, dtype=np.float32)

    # ---- host-side O(D^2) prep (f64) ----
    q = z_curr.reshape(-1).astype(np.float64) @ Wq.T.astype(np.float64) + bq
    A = np.zeros((D2, HEADS), np.float64)
    c = np.zeros(HEADS, np.float64)
    for h in range(HEADS):
        sl = slice(h * 2 * HD, (h + 1) * 2 * HD)
        A[:, h] = Wk[sl, :].T.astype(np.float64) @ q[sl]
        c[h] = bk[sl].astype(np.float64) @ q[sl]
    relflat = rel_bias.reshape(2 * REL_MAX + 1, D2).astype(np.float64)
    rb = np.stack(
        [
            relflat[:, h * 2 * HD : (h + 1) * 2 * HD] @ q[h * 2 * HD : (h + 1) * 2 * HD]
            for h in range(HEADS)
        ],
        axis=1,
    )  # [129, 8]
    idx = np.clip(
        curr_pos - L_TOTAL + np.arange(L_TOTAL) + REL_MAX, 0, 2 * REL_MAX
    ).astype(np.int64)

    z8 = np.clip(z_past.reshape(L_TOTAL, D2), -240.0, 240.0).astype(FP8)
    A8 = np.clip(A, -240.0, 240.0).astype(np.float32).astype(FP8)
    a_dr = np.zeros((128, 2, 2, 16), FP8)
    a_dr[:, :, :, 0:8] = A8.reshape(2, 2, 128, HEADS).transpose(2, 0, 1, 3)

    in_maps = []
    lnfs = []
    for core in range(N_CORES):
        zc = z8[core * L_SHARD : (core + 1) * L_SHARD]
        # zt[sup, p, blk, cpair, d, l] = zc[512*(4*sup+blk) + l, 256*cpair + 128d + p]
        zt = np.ascontiguousarray(
            zc.reshape(N_SUPER, BLK_PER_SUPER, 512, 2, 2, 128).transpose(
                0, 5, 1, 3, 4, 2
            )
        )
        # zn[sup, p, blk, s, d, f] = zc[512*(4*sup+blk) + 256s + 128d + p, f]
        zn = np.ascontiguousarray(
            zc.reshape(N_SUPER, BLK_PER_SUPER, 2, 2, 128, D2).transpose(
                0, 4, 1, 2, 3, 5
            )
        )
        idx_c = idx[core * L_SHARD : (core + 1) * L_SHARD]
        base = int(np.bincount(idx_c, minlength=2 * REL_MAX + 1).argmax())
        # cb and the rel corrections are PRE-scale here (the exp applies
        # SCALE to the whole accumulated score)
        cbr = np.zeros((1, 4, 16), BF16)
        cbr[0, :, 0:8] = (c + rb[base]).astype(np.float32).astype(BF16)
        lnfull = (rb[idx_c] - rb[base]).astype(np.float32)  # [L_SHARD, 8]
        lnfs.append(lnfull)
        in_maps.append(
            {
                "zt": zt,
                "zn": zn,
                "a_dr": a_dr,
                "cbr": cbr,
                "lnf": lnfull,  # reshaped below once full_fac is known
            }
        )

    # fast path: rel corrections vanish outside block 0 on every core
    full_fac = any(np.any(lnf[512:] != 0.0) for lnf in lnfs)
    n_lnf = N_BLOCKS if full_fac else 1
    for m in in_maps:
        lnf = m["lnf"][: n_lnf * 512]
        # [n_lnf*512, 8] -> [128(p), n_lnf, 4(q), 8]: row l = 512*b + 128*q + p
        m["lnf"] = np.ascontiguousarray(
            lnf.reshape(n_lnf, 4, 128, HEADS).transpose(2, 0, 1, 3)
        )
    nc = _get_program(full_fac)
    res = run_bass_kernel_spmd(
        nc, in_maps, list(range(N_CORES)), trace=TRACE, **TRACE_KW
    )
    if TRACE:
        kernel.last_result = res

    U = np.zeros((HEADS, D2), np.float64)
    S = np.zeros(HEADS, np.float64)
    for r in res.results:
        o = np.asarray(r["out"], dtype=np.float64)
        U += o[:, 0:512] + o[:, 512:1024]
        S += o[:, 1024]

    hvec = np.zeros(D2, np.float64)
    for h in range(HEADS):
        sl = slice(h * 2 * HD, (h + 1) * 2 * HD)
        hvec[sl] = Wv[sl, :].astype(np.float64) @ (U[h] / S[h]) + bv[sl]
    out = hvec @ Wo.T.astype(np.float64) + bo
    return out.reshape(DIM, 2).astype(np.float32)


# revision 13
# speedup vs baseline: 1.1116x; 1.1116x over previous
"""Trainium2 Bass kernel for nn_MultiHeadModulator (8-core SPMD).

Math reformulation (exact): with a single query q = Wq@z_curr+bq,
  - dot scores:  score[l,h] = z[l]·A[:,h] + c[h],   A[:,h] = Wk[hb,:]^T @ q[hb]
  - rel scores fold into a per-(l,h) additive bias known on the host
  - value sum:   sum_l e[l,h]*v[l] = Wv @ (sum_l e[l,h]*z[l]) + (sum_l e[l,h])*bv
so the device only computes, per L-shard:
  score^T = A^T z^T   (PE, fp8 DoubleRow),  e^T = exp(scale*score + c_h) * fac
  U[h,:] += e^T z     (PE, fp8 DoubleRow),  S[h] from exp's accum_out
and the host applies Wv/Wo and the softmax normalization to the tiny [8,512]
all-core sums.  Softmax runs without max-subtraction: scores are O(1) by
construction (validated |score| < 3).

Sharding: z_past split into 8 contiguous shards of 8192 rows, one per core.
The host ships each shard twice (feature-major for scores, row-major for U)
in fp8, pre-packed for DoubleRow access patterns (the dual layout costs 2x
HBM but avoids any on-chip transpose of z; only the tiny e^T [8,512] tiles
get PE-transposed per block).  Alternatives measured and rejected: z as
DoubleRow LDWEIGHTS (weight streaming is 3x slower than the moving path),
PE/DVE/XBAR transposes of z (all cost more than the dual shipment).

Scheduling notes (hard-won):
  - z streams as 16 half-super chunks (512 KB) zt/zn interleaved on the sync
    HWDGE ring, dispatched before the compute loop in consumption order.
    Coarse 1 MB chunks make the PE wait on super boundaries, which de-ramps
    the PE clock (p-state) and doubles matmul issue time for ~3 us after.
  - consts ride the scalar ring; gpsimd/SWDGE first-byte is ~10 us.
  - per-engine program order is a hard serialization (tick-counter sems):
    U(b-1) is emitted after score(b) so the PE covers the exp/cast chain
    latency with useful work.  Deeper lookahead (score(b+2) style) creates
    long-range tick waits between ACT and PE and collapses into a ~1.5
    us/block cross-engine limit cycle - do not.
  - weight-side DoubleRow LDWEIGHTS requires the pair-dim step to be a
    multiple of 16 elements (hence the [.., 16]-padded weight layouts).
  - nc.vector.tensor_tensor_reduce crashes on HW (fine in CoreSim); S uses
    the exp's accum_out for uncorrected blocks + a DVE reduce for block 0.
  - PSUM budget (8 banks): 3x score + 3x e-transpose + 2x U accumulator
    (even/odd blocks, so the even half's flush copy hides under block 15).
"""

import numpy as np
import ml_dtypes

import concourse.bass as bass  # noqa: F401  (engine namespaces live on the nc)
import concourse.mybir as mybir
import concourse.tile as tile
from concourse import bacc
from concourse.bass_utils import run_bass_kernel_spmd

HEADS = 8
REL_MAX = 64
DIM = 256
D2 = 512                      # flattened real feature dim
HD = DIM // HEADS             # 32 complex => 64 reals per head block
L_TOTAL = 65536
N_CORES = 8
L_SHARD = L_TOTAL // N_CORES  # 8192
N_BLOCKS = L_SHARD // 512     # 16 blocks of 512 rows
BLK_PER_SUPER = 4             # host packing granularity (1 MB supers)
N_SUPER = N_BLOCKS // BLK_PER_SUPER
SCALE = 1.0 / np.sqrt(HD)

FP8 = ml_dtypes.float8_e4m3   # == mybir.dt.float8e4 (trainium E4M3, max 240)
BF16 = ml_dtypes.bfloat16

TRACE = False                 # test.py can flip this for profiling runs
TRACE_KW = {}

_cached = {}


def _build_program(full_fac: bool):
    nc = bacc.Bacc(
        "TRN2", target_bir_lowering=False, debug=False, num_devices=N_CORES
    )
    DR = mybir.MatmulPerfMode.DoubleRow
    f8 = mybir.dt.float8e4

    ZT = nc.dram_tensor(
        "zt", [N_SUPER, 128, BLK_PER_SUPER, 2, 2, 512], f8, kind="ExternalInput"
    )
    ZN = nc.dram_tensor(
        "zn", [N_SUPER, 128, BLK_PER_SUPER, 2, 2, 512], f8, kind="ExternalInput"
    )
    AT = nc.dram_tensor("a_dr", [128, 2, 2, 16], f8, kind="ExternalInput")
    FAC = nc.dram_tensor(
        "fac", [8, L_SHARD if full_fac else 512], mybir.dt.bfloat16,
        kind="ExternalInput",
    )
    CB = nc.dram_tensor("cb", [8, 1], mybir.dt.float32, kind="ExternalInput")
    IDENT = nc.dram_tensor("ident", [8, 8], mybir.dt.bfloat16, kind="ExternalInput")
    # output: cols 0..511 = U (even blocks), 512..1023 = U (odd blocks),
    # cols 1024..1039 = per-block S partials; host sums the two U halves
    OUT = nc.dram_tensor("out", [8, 1040], mybir.dt.float32, kind="ExternalOutput")

    with tile.TileContext(nc) as tc:
        with (
            tc.tile_pool(name="zt", bufs=N_BLOCKS // 2) as zt_pool,
            tc.tile_pool(name="zn", bufs=N_BLOCKS // 2) as zn_pool,
            tc.tile_pool(name="consts", bufs=1) as const_pool,
            tc.tile_pool(name="et", bufs=6) as et_pool,
            tc.tile_pool(name="e8", bufs=6) as e8_pool,
            tc.tile_pool(name="outs", bufs=1) as out_pool,
            tc.tile_pool(name="ps_sc", bufs=3, space="PSUM") as sc_pool,
            tc.tile_pool(name="ps_etp", bufs=3, space="PSUM") as etp_pool,
            tc.tile_pool(name="ps_acc", bufs=1, space="PSUM") as acc_pool,
        ):
            # consts ride the scalar ring (tiny; the sync ring is fully
            # booked streaming z in consumption order)
            a_sb = const_pool.tile([128, 2, 2, 16], f8)
            nc.scalar.dma_start(a_sb[:], AT[:])
            cb_sb = const_pool.tile([8, 1], mybir.dt.float32)
            nc.scalar.dma_start(cb_sb[:], CB[:])
            id_sb = const_pool.tile([8, 8], mybir.dt.bfloat16)
            nc.scalar.dma_start(id_sb[:], IDENT[:])
            fac_sb = const_pool.tile(
                [8, L_SHARD if full_fac else 512], mybir.dt.bfloat16
            )
            nc.scalar.dma_start(fac_sb[:], FAC[:])

            # z streams as 16 half-super chunks (2 blocks each), zt/zn
            # interleaved, so the PE never waits on a coarse 1 MB completion
            # semaphore at super boundaries
            N_HALF = N_BLOCKS // 2
            zt_tiles = [None] * N_HALF
            zn_tiles = [None] * N_HALF
            for h in range(N_HALF):
                sup, hb = divmod(h, 2)
                zt_h = zt_pool.tile([128, 2, 2, 2, 512], f8, tag="zt_h")
                nc.sync.dma_start(zt_h[:], ZT[sup][:, 2 * hb : 2 * hb + 2])
                zt_tiles[h] = zt_h
                zn_h = zn_pool.tile([128, 2, 2, 2, 512], f8, tag="zn_h")
                nc.sync.dma_start(zn_h[:], ZN[sup][:, 2 * hb : 2 * hb + 2])
                zn_tiles[h] = zn_h

            # two U accumulators (even/odd blocks) so the even half's
            # PSUM->SBUF copy overlaps the last block's compute
            u_even = acc_pool.tile([8, 512], mybir.dt.float32, tag="u0")
            u_odd = acc_pool.tile([8, 512], mybir.dt.float32, tag="u1")
            u_ps = [u_even, u_odd]
            out_sb = out_pool.tile([8, 1040], mybir.dt.float32)

            e8s = [None] * N_BLOCKS

            def emit_u(b):
                zn_t = zn_tiles[b // 2][:, b % 2]
                e8 = e8s[b]
                par = b & 1
                for s in range(2):
                    nc.tensor.matmul(
                        u_ps[par][:],
                        e8[:, 2 * s : 2 * s + 2, 0:8],
                        zn_t[:, s],
                        start=(b == par and s == 0),
                        stop=(b >= N_BLOCKS - 2 and s == 1),
                        perf_mode=DR,
                    )

            for b in range(N_BLOCKS):
                zt_t = zt_tiles[b // 2][:, b % 2]

                # score^T[h, l] for this block's 512 rows, K=512 via 2x DR
                sc = sc_pool.tile([8, 512], mybir.dt.float32, tag="sc")
                for cpair in range(2):
                    nc.tensor.matmul(
                        sc[:],
                        a_sb[:, cpair, :, 0:8],
                        zt_t[:, cpair],
                        start=(cpair == 0),
                        stop=(cpair == 1),
                        perf_mode=DR,
                    )

                # U(b-1) fills the PE while block b's exp/cast chain runs on
                # ACT/DVE (1-block skew only: longer-range tick waits stall)
                if b > 0:
                    emit_u(b - 1)

                et = et_pool.tile([8, 512], mybir.dt.bfloat16, tag="et")
                # for fac==1 blocks, S comes free from the exp's accum_out
                accum = (
                    {}
                    if (full_fac or b == 0)
                    else {"accum_out": out_sb[:, 1024 + b : 1025 + b]}
                )
                nc.scalar.activation(
                    et[:],
                    sc[:],
                    mybir.ActivationFunctionType.Exp,
                    bias=cb_sb[:, 0:1],
                    scale=float(SCALE),
                    **accum,
                )
                # rel-bias correction factors: only block 0 deviates from 1
                # in the common curr_pos regime (full_fac covers the rest)
                if full_fac or b == 0:
                    etc = et_pool.tile([8, 512], mybir.dt.bfloat16, tag="etc")
                    nc.vector.tensor_mul(
                        etc[:], et[:], fac_sb[:, 512 * b : 512 * (b + 1)]
                    )
                    # S for corrected blocks: one DVE free-axis reduction
                    nc.vector.tensor_reduce(
                        out_sb[:, 1024 + b : 1025 + b],
                        etc[:],
                        axis=mybir.AxisListType.X,
                        op=mybir.AluOpType.add,
                    )
                else:
                    etc = et

                # transpose e^T -> e[l,h] in 4x [8,128] chunks (PE via identity)
                etp = etp_pool.tile([128, 4, 8], mybir.dt.bfloat16)
                for quad in range(4):
                    nc.tensor.transpose(
                        etp[:, quad],
                        etc[:, 128 * quad : 128 * (quad + 1)],
                        id_sb[:],
                    )
                e8 = e8_pool.tile([128, 4, 16], f8)
                nc.vector.tensor_copy(e8[:, :, 0:8], etp[:])
                e8s[b] = e8

                if b == N_BLOCKS - 1:
                    # even-half flush overlaps block 15's compute (DVE copy)
                    nc.vector.tensor_copy(out_sb[:, 0:512], u_ps[0][:])
                    nc.sync.dma_start(OUT[:, 0:512], out_sb[:, 0:512])
            emit_u(N_BLOCKS - 1)

            # final (odd) U copy on ACT (idle after exp(15)), S rides along
            nc.scalar.copy(out_sb[:, 512:1024], u_ps[1][:])
            nc.sync.dma_start(OUT[:, 512:1040], out_sb[:, 512:1040])

    nc.compile()
    return nc


def _get_program(full_fac: bool):
    if full_fac not in _cached:
        _cached[full_fac] = _build_program(full_fac)
    return _cached[full_fac]


def kernel(curr_pos, z_curr, z_past, Wq, bq, Wk, bk, Wv, bv, Wo, bo, rel_bias):
    curr_pos = int(np.asarray(curr_pos))
    z_curr = np.asarray(z_curr, dtype=np.float32)
    z_past = np.asarray(z_past, dtype=np.float32)
    Wq = np.asarray(Wq, dtype=np.float32)
    bq = np.asarray(bq, dtype=np.float32)
    Wk = np.asarray(Wk, dtype=np.float32)
    bk = np.asarray(bk, dtype=np.float32)
    Wv = np.asarray(Wv, dtype=np.float32)
    bv = np.asarray(bv, dtype=np.float32)
    Wo = np.asarray(Wo, dtype=np.float32)
    bo = np.asarray(bo, dtype=np.float32)
    rel_bias = np.asarray(rel_bias, dtype=np.float32)

    # ---- host-side O(D^2) prep (f64) ----
    q = z_curr.reshape(-1).astype(np.float64) @ Wq.T.astype(np.float64) + bq
    A = np.zeros((D2, HEADS), np.float64)
    c = np.zeros(HEADS, np.float64)
    for h in range(HEADS):
        sl = slice(h * 2 * HD, (h + 1) * 2 * HD)
        A[:, h] = Wk[sl, :].T.astype(np.float64) @ q[sl]
        c[h] = bk[sl].astype(np.float64) @ q[sl]
    relflat = rel_bias.reshape(2 * REL_MAX + 1, D2).astype(np.float64)
    rb = np.stack(
        [
            relflat[:, h * 2 * HD : (h + 1) * 2 * HD] @ q[h * 2 * HD : (h + 1) * 2 * HD]
            for h in range(HEADS)
        ],
        axis=1,
    )  # [129, 8]
    idx = np.clip(
        curr_pos - L_TOTAL + np.arange(L_TOTAL) + REL_MAX, 0, 2 * REL_MAX
    ).astype(np.int64)

    z8 = np.clip(z_past.reshape(L_TOTAL, D2), -240.0, 240.0).astype(FP8)
    A8 = np.clip(A, -240.0, 240.0).astype(np.float32).astype(FP8)
    a_dr = np.zeros((128, 2, 2, 16), FP8)
    a_dr[:, :, :, 0:8] = A8.reshape(2, 2, 128, HEADS).transpose(2, 0, 1, 3)

    ident = np.eye(8, dtype=BF16)

    in_maps = []
    facs = []
    for core in range(N_CORES):
        zc = z8[core * L_SHARD : (core + 1) * L_SHARD]
        # zt[sup, p, blk, cpair, d, l] = zc[512*(4*sup+blk) + l, 256*cpair + 128d + p]
        zt = np.ascontiguousarray(
            zc.reshape(N_SUPER, BLK_PER_SUPER, 512, 2, 2, 128).transpose(
                0, 5, 1, 3, 4, 2
            )
        )
        # zn[sup, p, blk, s, d, f] = zc[512*(4*sup+blk) + 256s + 128d + p, f]
        zn = np.ascontiguousarray(
            zc.reshape(N_SUPER, BLK_PER_SUPER, 2, 2, 128, D2).transpose(
                0, 4, 1, 2, 3, 5
            )
        )
        idx_c = idx[core * L_SHARD : (core + 1) * L_SHARD]
        base = int(np.bincount(idx_c, minlength=2 * REL_MAX + 1).argmax())
        cb = ((c + rb[base]) * SCALE).astype(np.float32).reshape(HEADS, 1)
        fac = np.ascontiguousarray(
            np.exp((rb[idx_c] - rb[base]) * SCALE).T.astype(BF16)
        )
        facs.append(fac)
        in_maps.append(
            {
                "zt": zt,
                "zn": zn,
                "a_dr": a_dr,
                "fac": fac,
                "cb": cb,
                "ident": ident,
            }
        )

    # fast path: correction factors are 1.0 outside block 0 on every core
    full_fac = any(
        not np.all(f[:, 512:] == np.asarray(1.0, BF16)) for f in facs
    )
    if not full_fac:
        for m in in_maps:
            m["fac"] = np.ascontiguousarray(m["fac"][:, 0:512])
    nc = _get_program(full_fac)
    res = run_bass_kernel_spmd(
        nc, in_maps, list(range(N_CORES)), trace=TRACE, **TRACE_KW
    )
    if TRACE:
        kernel.last_result = res

    U = np.zeros((HEADS, D2), np.float64)
    S = np.zeros(HEADS, np.float64)
    for r in res.results:
        o = np.asarray(r["out"], dtype=np.float64)
        U += o[:, 0:512] + o[:, 512:1024]
        S += o[:, 1024:1040].sum(axis=1)

    hvec = np.zeros(D2, np.float64)
    for h in range(HEADS):
        sl = slice(h * 2 * HD, (h + 1) * 2 * HD)
        hvec[sl] = Wv[sl, :].astype(np.float64) @ (U[h] / S[h]) + bv[sl]
    out = hvec @ Wo.T.astype(np.float64) + bo
    return out.reshape(DIM, 2).astype(np.float32)


# revision 14
# speedup vs baseline: 1.2576x; 1.1313x over previous
"""Trainium2 Bass kernel for nn_MultiHeadModulator (8-core SPMD).

Math reformulation (exact): with a single query q = Wq@z_curr+bq,
  - dot scores:  score[l,h] = z[l]·A[:,h] + c[h],   A[:,h] = Wk[hb,:]^T @ q[hb]
  - rel scores fold into a per-(l,h) additive bias known on the host
  - value sum:   sum_l e[l,h]*v[l] = Wv @ (sum_l e[l,h]*z[l]) + (sum_l e[l,h])*bv
so the device only computes, per L-shard:
  score^T = A^T z^T   (PE, fp8 DoubleRow),  e^T = exp(scale*score + c_h) * fac
  U[h,:] += e^T z     (PE, fp8 DoubleRow),  S[h] from exp's accum_out
and the host applies Wv/Wo and the softmax normalization to the tiny [8,512]
all-core sums.  Softmax runs without max-subtraction: scores are O(1) by
construction (validated |score| < 3).

Sharding: z_past split into 8 contiguous shards of 8192 rows, one per core.
The host ships each shard twice (feature-major for scores, row-major for U)
in fp8, pre-packed for DoubleRow access patterns (the dual layout costs 2x
HBM but avoids any on-chip transpose of z; only the tiny e^T [8,512] tiles
get PE-transposed per block).  Alternatives measured and rejected: z as
DoubleRow LDWEIGHTS (weight streaming is 3x slower than the moving path),
PE/DVE/XBAR transposes of z (all cost more than the dual shipment).

Scheduling notes (hard-won):
  - z streams as 16 half-super chunks (512 KB) zt/zn interleaved on the sync
    HWDGE ring, dispatched before the compute loop in consumption order.
    Coarse 1 MB chunks make the PE wait on super boundaries, which de-ramps
    the PE clock (p-state) and doubles matmul issue time for ~3 us after.
  - consts ride the scalar ring; gpsimd/SWDGE first-byte is ~10 us.
  - per-engine program order is a hard serialization (tick-counter sems):
    U(b-1) is emitted after score(b) so the PE covers the exp/cast chain
    latency with useful work.  Deeper lookahead (score(b+2) style) creates
    long-range tick waits between ACT and PE and collapses into a ~1.5
    us/block cross-engine limit cycle - do not.
  - weight-side DoubleRow LDWEIGHTS requires the pair-dim step to be a
    multiple of 16 elements (hence the [.., 16]-padded weight layouts).
  - nc.vector.tensor_tensor_reduce crashes on HW (fine in CoreSim); S uses
    the exp's accum_out for uncorrected blocks + a DVE reduce for block 0.
  - PSUM budget (8 banks): 3x score + 3x e-transpose + 2x U accumulator
    (even/odd blocks, so the even half's flush copy hides under block 15).
"""

import numpy as np
import ml_dtypes

import concourse.bass as bass  # noqa: F401  (engine namespaces live on the nc)
import concourse.mybir as mybir
import concourse.tile as tile
from concourse import bacc
from concourse.bass_utils import run_bass_kernel_spmd

HEADS = 8
REL_MAX = 64
DIM = 256
D2 = 512                      # flattened real feature dim
HD = DIM // HEADS             # 32 complex => 64 reals per head block
L_TOTAL = 65536
N_CORES = 8
L_SHARD = L_TOTAL // N_CORES  # 8192
N_BLOCKS = L_SHARD // 512     # 16 blocks of 512 rows
BLK_PER_SUPER = 4             # host packing granularity (1 MB supers)
N_SUPER = N_BLOCKS // BLK_PER_SUPER
SCALE = 1.0 / np.sqrt(HD)

FP8 = ml_dtypes.float8_e4m3   # == mybir.dt.float8e4 (trainium E4M3, max 240)
BF16 = ml_dtypes.bfloat16

TRACE = False                 # test.py can flip this for profiling runs
TRACE_KW = {}

_cached = {}


def _build_program(full_fac: bool):
    nc = bacc.Bacc(
        "TRN2", target_bir_lowering=False, debug=False, num_devices=N_CORES
    )
    DR = mybir.MatmulPerfMode.DoubleRow
    f8 = mybir.dt.float8e4

    ZT = nc.dram_tensor(
        "zt", [N_SUPER, 128, BLK_PER_SUPER, 2, 2, 512], f8, kind="ExternalInput"
    )
    ZN = nc.dram_tensor(
        "zn", [N_SUPER, 128, BLK_PER_SUPER, 2, 2, 512], f8, kind="ExternalInput"
    )
    AT = nc.dram_tensor("a_dr", [128, 2, 2, 16], f8, kind="ExternalInput")
    FAC = nc.dram_tensor(
        "fac", [8, L_SHARD if full_fac else 512], mybir.dt.bfloat16,
        kind="ExternalInput",
    )
    CB = nc.dram_tensor("cb", [8, 1], mybir.dt.float32, kind="ExternalInput")
    IDENT = nc.dram_tensor("ident", [8, 8], mybir.dt.bfloat16, kind="ExternalInput")
    # output: cols 0..511 = U (even blocks), 512..1023 = U (odd blocks),
    # cols 1024..1039 = per-block S partials; host sums the two U halves
    OUT = nc.dram_tensor("out", [8, 1040], mybir.dt.float32, kind="ExternalOutput")

    with tile.TileContext(nc) as tc:
        with (
            tc.tile_pool(name="zt", bufs=N_BLOCKS // 2) as zt_pool,
            tc.tile_pool(name="zn", bufs=N_BLOCKS // 2) as zn_pool,
            tc.tile_pool(name="consts", bufs=1) as const_pool,
            tc.tile_pool(name="et", bufs=6) as et_pool,
            tc.tile_pool(name="e8", bufs=6) as e8_pool,
            tc.tile_pool(name="outs", bufs=1) as out_pool,
            tc.tile_pool(name="ps_sc", bufs=2, space="PSUM") as sc_pool,
            tc.tile_pool(name="ps_etp", bufs=3, space="PSUM") as etp_pool,
            tc.tile_pool(name="ps_acc", bufs=1, space="PSUM") as acc_pool,
        ):
            # a_dr rides the scalar ring (tiny, lands early in parallel with
            # the sync ring's zt0); ones are memset on the idle gpsimd
            a_sb = const_pool.tile([128, 2, 2, 16], f8)
            nc.scalar.dma_start(a_sb[:], AT[:])
            ones8 = const_pool.tile([128, 2, 16], f8)
            nc.gpsimd.memset(ones8[:], 1.0)

            zt_tiles = [None] * N_SUPER
            zn_tiles = [None] * N_SUPER

            def load_super(sup):
                zt_s = zt_pool.tile([128, BLK_PER_SUPER, 2, 2, 512], f8, tag="zt_s")
                zn_s = zn_pool.tile([128, BLK_PER_SUPER, 2, 2, 512], f8, tag="zn_s")
                nc.sync.dma_start(zt_s[:], ZT[sup])
                nc.sync.dma_start(zn_s[:], ZN[sup])
                zt_tiles[sup] = zt_s
                zn_tiles[sup] = zn_s

            load_super(0)
            cb_sb = const_pool.tile([8, 1], mybir.dt.float32)
            nc.sync.dma_start(cb_sb[:], CB[:])
            id_sb = const_pool.tile([8, 8], mybir.dt.bfloat16)
            nc.sync.dma_start(id_sb[:], IDENT[:])
            lnf_sb = const_pool.tile(
                [8, L_SHARD if full_fac else 512], mybir.dt.bfloat16
            )
            nc.sync.dma_start(lnf_sb[:], FAC[:])
            for sup in range(1, N_SUPER):
                load_super(sup)

            # two U accumulators (even/odd blocks) so the even half's
            # PSUM->SBUF copy overlaps the last block's compute
            u_even = acc_pool.tile([8, 512], mybir.dt.float32, tag="u0")
            u_odd = acc_pool.tile([8, 512], mybir.dt.float32, tag="u1")
            u_ps = [u_even, u_odd]
            s_ps = acc_pool.tile([8, 16], mybir.dt.float32, tag="s")
            out_sb = out_pool.tile([8, 1040], mybir.dt.float32)

            scbs = [None] * N_BLOCKS
            e8s = [None] * N_BLOCKS

            def emit_score(b):
                sup, blk = divmod(b, BLK_PER_SUPER)
                zt_t = zt_tiles[sup][:, blk]
                sc = sc_pool.tile([8, 512], mybir.dt.float32, tag="sc")
                for cpair in range(2):
                    nc.tensor.matmul(
                        sc[:],
                        a_sb[:, cpair, :, 0:8],
                        zt_t[:, cpair],
                        start=(cpair == 0),
                        stop=(cpair == 1),
                        perf_mode=DR,
                    )
                # DVE fuses the PSUM read, +cb bias, and bf16 downcast; the
                # rel-bias correction is a second DVE add (block 0 only in
                # the common curr_pos regime)
                scb = et_pool.tile([8, 512], mybir.dt.bfloat16, tag="scb")
                nc.vector.tensor_scalar_add(scb[:], sc[:], cb_sb[:, 0:1])
                if full_fac or b == 0:
                    nc.vector.tensor_add(
                        scb[:], scb[:], lnf_sb[:, 512 * b : 512 * (b + 1)]
                    )
                scbs[b] = scb

            def emit_tp_exp(b):
                # transpose the BIASED SCORES (not the exps): exp then runs
                # on 128 partitions (~30x less ACT time) and emits the fp8
                # U weights directly - no DVE cast, no accumulator reads
                scb = scbs[b]
                stp = etp_pool.tile([128, 4, 8], mybir.dt.bfloat16)
                for quad in range(4):
                    nc.tensor.transpose(
                        stp[:, quad],
                        scb[:, 128 * quad : 128 * (quad + 1)],
                        id_sb[:],
                    )
                e8 = e8_pool.tile([128, 4, 16], f8, tag="e8")
                nc.scalar.activation(
                    e8[:, :, 0:8],
                    stp[:],
                    mybir.ActivationFunctionType.Exp,
                    scale=float(SCALE),
                )
                e8s[b] = e8

            def emit_u(b):
                sup, blk = divmod(b, BLK_PER_SUPER)
                zn_t = zn_tiles[sup][:, blk]
                e8 = e8s[b]
                par = b & 1
                for srow in range(2):
                    nc.tensor.matmul(
                        u_ps[par][:],
                        e8[:, 2 * srow : 2 * srow + 2, 0:8],
                        zn_t[:, srow],
                        start=(b == par and srow == 0),
                        stop=(b >= N_BLOCKS - 2 and srow == 1),
                        perf_mode=DR,
                    )
                    # S rides the already-loaded weights: one extra column
                    nc.tensor.matmul(
                        s_ps[:, 0:1],
                        e8[:, 2 * srow : 2 * srow + 2, 0:8],
                        ones8[:, :, 0:1],
                        start=(b == 0 and srow == 0),
                        stop=(b == N_BLOCKS - 1 and srow == 1),
                        perf_mode=DR,
                    )

            # per-iteration PE order score(b), TP(b-1), U(b-2): every
            # cross-engine dep is at most 2 blocks old (long-range tick
            # waits collapse the pipeline into a cross-engine limit cycle)
            for b in range(N_BLOCKS):
                emit_score(b)
                if b >= 1:
                    emit_tp_exp(b - 1)
                if b >= 2:
                    emit_u(b - 2)
            emit_tp_exp(N_BLOCKS - 1)
            emit_u(N_BLOCKS - 2)
            # even-half flush on ACT (idle) overlaps the last U matmuls
            nc.scalar.copy(out_sb[:, 0:512], u_ps[0][:])
            nc.sync.dma_start(OUT[:, 0:512], out_sb[:, 0:512])
            emit_u(N_BLOCKS - 1)

            # final (odd) U + S copies on ACT, idle at kernel end
            nc.scalar.copy(out_sb[:, 512:1024], u_ps[1][:])
            nc.scalar.copy(out_sb[:, 1024:1040], s_ps[:])
            nc.sync.dma_start(OUT[:, 512:1040], out_sb[:, 512:1040])

    nc.compile()
    return nc


def _get_program(full_fac: bool):
    if full_fac not in _cached:
        _cached[full_fac] = _build_program(full_fac)
    return _cached[full_fac]


def kernel(curr_pos, z_curr, z_past, Wq, bq, Wk, bk, Wv, bv, Wo, bo, rel_bias):
    curr_pos = int(np.asarray(curr_pos))
    z_curr = np.asarray(z_curr, dtype=np.float32)
    z_past = np.asarray(z_past, dtype=np.float32)
    Wq = np.asarray(Wq, dtype=np.float32)
    bq = np.asarray(bq, dtype=np.float32)
    Wk = np.asarray(Wk, dtype=np.float32)
    bk = np.asarray(bk, dtype=np.float32)
    Wv = np.asarray(Wv, dtype=np.float32)
    bv = np.asarray(bv, dtype=np.float32)
    Wo = np.asarray(Wo, dtype=np.float32)
    bo = np.asarray(bo, dtype=np.float32)
    rel_bias = np.asarray(rel_bias, dtype=np.float32)

    # ---- host-side O(D^2) prep (f64) ----
    q = z_curr.reshape(-1).astype(np.float64) @ Wq.T.astype(np.float64) + bq
    A = np.zeros((D2, HEADS), np.float64)
    c = np.zeros(HEADS, np.float64)
    for h in range(HEADS):
        sl = slice(h * 2 * HD, (h + 1) * 2 * HD)
        A[:, h] = Wk[sl, :].T.astype(np.float64) @ q[sl]
        c[h] = bk[sl].astype(np.float64) @ q[sl]
    relflat = rel_bias.reshape(2 * REL_MAX + 1, D2).astype(np.float64)
    rb = np.stack(
        [
            relflat[:, h * 2 * HD : (h + 1) * 2 * HD] @ q[h * 2 * HD : (h + 1) * 2 * HD]
            for h in range(HEADS)
        ],
        axis=1,
    )  # [129, 8]
    idx = np.clip(
        curr_pos - L_TOTAL + np.arange(L_TOTAL) + REL_MAX, 0, 2 * REL_MAX
    ).astype(np.int64)

    z8 = np.clip(z_past.reshape(L_TOTAL, D2), -240.0, 240.0).astype(FP8)
    A8 = np.clip(A, -240.0, 240.0).astype(np.float32).astype(FP8)
    a_dr = np.zeros((128, 2, 2, 16), FP8)
    a_dr[:, :, :, 0:8] = A8.reshape(2, 2, 128, HEADS).transpose(2, 0, 1, 3)

    ident = np.eye(8, dtype=BF16)

    in_maps = []
    facs = []
    for core in range(N_CORES):
        zc = z8[core * L_SHARD : (core + 1) * L_SHARD]
        # zt[sup, p, blk, cpair, d, l] = zc[512*(4*sup+blk) + l, 256*cpair + 128d + p]
        zt = np.ascontiguousarray(
            zc.reshape(N_SUPER, BLK_PER_SUPER, 512, 2, 2, 128).transpose(
                0, 5, 1, 3, 4, 2
            )
        )
        # zn[sup, p, blk, s, d, f] = zc[512*(4*sup+blk) + 256s + 128d + p, f]
        zn = np.ascontiguousarray(
            zc.reshape(N_SUPER, BLK_PER_SUPER, 2, 2, 128, D2).transpose(
                0, 4, 1, 2, 3, 5
            )
        )
        idx_c = idx[core * L_SHARD : (core + 1) * L_SHARD]
        base = int(np.bincount(idx_c, minlength=2 * REL_MAX + 1).argmax())
        cb = (c + rb[base]).astype(np.float32).reshape(HEADS, 1)
        fac = np.ascontiguousarray(
            (rb[idx_c] - rb[base]).T.astype(BF16)
        )
        facs.append(fac)
        in_maps.append(
            {
                "zt": zt,
                "zn": zn,
                "a_dr": a_dr,
                "fac": fac,
                "cb": cb,
                "ident": ident,
            }
        )

    # fast path: rel corrections vanish outside block 0 on every core
    full_fac = any(not np.all(f[:, 512:] == np.asarray(0.0, BF16)) for f in facs)
    if not full_fac:
        for m in in_maps:
            m["fac"] = np.ascontiguousarray(m["fac"][:, 0:512])
    nc = _get_program(full_fac)
    res = run_bass_kernel_spmd(
        nc, in_maps, list(range(N_CORES)), trace=TRACE, **TRACE_KW
    )
    if TRACE:
        kernel.last_result = res

    U = np.zeros((HEADS, D2), np.float64)
    S = np.zeros(HEADS, np.float64)
    for r in res.results:
        o = np.asarray(r["out"], dtype=np.float64)
        U += o[:, 0:512] + o[:, 512:1024]
        S += o[:, 1024]

    hvec = np.zeros(D2, np.float64)
    for h in range(HEADS):
        sl = slice(h * 2 * HD, (h + 1) * 2 * HD)
        hvec[sl] = Wv[sl, :].astype(np.float64) @ (U[h] / S[h]) + bv[sl]
    out = hvec @ Wo.T.astype(np.float64) + bo
    return out.reshape(DIM, 2).astype(np.float32)


# revision 15
# speedup vs baseline: 1.2843x; 1.0212x over previous
"""Trainium2 Bass kernel for nn_MultiHeadModulator (8-core SPMD).

Math reformulation (exact): with a single query q = Wq@z_curr+bq,
  - dot scores:  score[l,h] = z[l]·A[:,h] + c[h],   A[:,h] = Wk[hb,:]^T @ q[hb]
  - rel scores fold into a per-(l,h) additive bias known on the host
  - value sum:   sum_l e[l,h]*v[l] = Wv @ (sum_l e[l,h]*z[l]) + (sum_l e[l,h])*bv
so the device only computes, per L-shard:
  score^T = A^T z^T   (PE, fp8 DoubleRow),  e^T = exp(scale*score + c_h) * fac
  U[h,:] += e^T z     (PE, fp8 DoubleRow),  S[h] from exp's accum_out
and the host applies Wv/Wo and the softmax normalization to the tiny [8,512]
all-core sums.  Softmax runs without max-subtraction: scores are O(1) by
construction (validated |score| < 3).

Sharding: z_past split into 8 contiguous shards of 8192 rows, one per core.
The host ships each shard twice (feature-major for scores, row-major for U)
in fp8, pre-packed for DoubleRow access patterns (the dual layout costs 2x
HBM but avoids any on-chip transpose of z; only the tiny e^T [8,512] tiles
get PE-transposed per block).

Measured: ~41.2 us HW exec (8 cores), rel err 5.8e-3 vs the f32 reference.
Roofline: ~23.5 us of per-core HBM traffic (8.4 MB @ 358 GB/s) + ~7.5 us
fixed NEFF preamble + ~3 us tail.

Scheduling notes (hard-won):
  - all bulk loads ride the sync HWDGE ring, dispatched before the compute
    loop in consumption order (zt0, zn0, zt1, ...); a_dr goes on the scalar
    ring in parallel.  Bulk DMAs emitted inside the block loop get
    interleaved AFTER exp instructions on the scalar sequencer and stall.
    gpsimd/SWDGE first-byte is ~10 us - never put early loads there.
  - weight-side DoubleRow LDWEIGHTS requires the pair-dim step to be a
    multiple of 16 elements (hence the [.., 16]-padded weight layouts).
  - nc.vector.tensor_tensor_reduce crashes on HW (fine in CoreSim); S uses
    the exp's accum_out for uncorrected blocks + a DVE reduce for block 0.
  - PSUM budget (8 banks): 4x score + 3x e-transpose + 1x U accumulator.
"""

import numpy as np
import ml_dtypes

import concourse.bass as bass  # noqa: F401  (engine namespaces live on the nc)
import concourse.mybir as mybir
import concourse.tile as tile
from concourse import bacc
from concourse.bass_utils import run_bass_kernel_spmd

HEADS = 8
REL_MAX = 64
DIM = 256
D2 = 512                      # flattened real feature dim
HD = DIM // HEADS             # 32 complex => 64 reals per head block
L_TOTAL = 65536
N_CORES = 8
L_SHARD = L_TOTAL // N_CORES  # 8192
N_BLOCKS = L_SHARD // 512     # 16 blocks of 512 rows
BLK_PER_SUPER = 4             # blocks per DMA (1 MB chunks)
N_SUPER = N_BLOCKS // BLK_PER_SUPER
SCALE = 1.0 / np.sqrt(HD)

FP8 = ml_dtypes.float8_e4m3   # == mybir.dt.float8e4 (trainium E4M3, max 240)
BF16 = ml_dtypes.bfloat16

TRACE = False                 # test.py can flip this for profiling runs
TRACE_KW = {}

_cached = {}


def _build_program(full_fac: bool):
    nc = bacc.Bacc(
        "TRN2", target_bir_lowering=False, debug=False, num_devices=N_CORES
    )
    DR = mybir.MatmulPerfMode.DoubleRow
    f8 = mybir.dt.float8e4

    ZT = nc.dram_tensor(
        "zt", [N_SUPER, 128, BLK_PER_SUPER, 2, 2, 512], f8, kind="ExternalInput"
    )
    ZN = nc.dram_tensor(
        "zn", [N_SUPER, 128, BLK_PER_SUPER, 2, 2, 512], f8, kind="ExternalInput"
    )
    AT = nc.dram_tensor("a_dr", [128, 2, 2, 16], f8, kind="ExternalInput")
    FAC = nc.dram_tensor(
        "fac", [8, L_SHARD if full_fac else 512], mybir.dt.bfloat16,
        kind="ExternalInput",
    )
    CB = nc.dram_tensor("cb", [8, 1], mybir.dt.float32, kind="ExternalInput")
    IDENT = nc.dram_tensor("ident", [8, 8], mybir.dt.bfloat16, kind="ExternalInput")
    # single output: cols 0..511 = U, cols 512..527 = per-block S partials
    OUT = nc.dram_tensor("out", [8, 528], mybir.dt.float32, kind="ExternalOutput")

    with tile.TileContext(nc) as tc:
        with (
            tc.tile_pool(name="zt", bufs=N_SUPER) as zt_pool,
            tc.tile_pool(name="zn", bufs=N_SUPER) as zn_pool,
            tc.tile_pool(name="consts", bufs=1) as const_pool,
            tc.tile_pool(name="et", bufs=6) as et_pool,
            tc.tile_pool(name="e8", bufs=6) as e8_pool,
            tc.tile_pool(name="outs", bufs=1) as out_pool,
            tc.tile_pool(name="ps_sc", bufs=4, space="PSUM") as sc_pool,
            tc.tile_pool(name="ps_etp", bufs=3, space="PSUM") as etp_pool,
            tc.tile_pool(name="ps_acc", bufs=1, space="PSUM") as acc_pool,
        ):
            # a_dr rides the scalar ring (tiny, lands early in parallel with
            # the sync ring's zt0)
            a_sb = const_pool.tile([128, 2, 2, 16], f8)
            nc.scalar.dma_start(a_sb[:], AT[:])

            zt_tiles = [None] * N_SUPER
            zn_tiles = [None] * N_SUPER

            def load_super(sup):
                zt_s = zt_pool.tile([128, BLK_PER_SUPER, 2, 2, 512], f8, tag="zt_s")
                zn_s = zn_pool.tile([128, BLK_PER_SUPER, 2, 2, 512], f8, tag="zn_s")
                nc.sync.dma_start(zt_s[:], ZT[sup])
                nc.sync.dma_start(zn_s[:], ZN[sup])
                zt_tiles[sup] = zt_s
                zn_tiles[sup] = zn_s

            load_super(0)
            cb_sb = const_pool.tile([8, 1], mybir.dt.float32)
            nc.sync.dma_start(cb_sb[:], CB[:])
            id_sb = const_pool.tile([8, 8], mybir.dt.bfloat16)
            nc.sync.dma_start(id_sb[:], IDENT[:])
            fac_sb = const_pool.tile(
                [8, L_SHARD if full_fac else 512], mybir.dt.bfloat16
            )
            nc.sync.dma_start(fac_sb[:], FAC[:])
            for sup in range(1, N_SUPER):
                load_super(sup)

            u_ps = acc_pool.tile([8, 512], mybir.dt.float32)
            out_sb = out_pool.tile([8, 528], mybir.dt.float32)

            for b in range(N_BLOCKS):
                sup, blk = divmod(b, BLK_PER_SUPER)
                zt_t = zt_tiles[sup][:, blk]
                zn_t = zn_tiles[sup][:, blk]

                # score^T[h, l] for this block's 512 rows, K=512 via 2x DoubleRow
                sc = sc_pool.tile([8, 512], mybir.dt.float32)
                for cpair in range(2):
                    nc.tensor.matmul(
                        sc[:],
                        a_sb[:, cpair, :, 0:8],
                        zt_t[:, cpair],
                        start=(cpair == 0),
                        stop=(cpair == 1),
                        perf_mode=DR,
                    )

                et = et_pool.tile([8, 512], mybir.dt.bfloat16, tag="et")
                # for fac==1 blocks, S comes free from the exp's accum_out
                accum = (
                    {}
                    if (full_fac or b == 0)
                    else {"accum_out": out_sb[:, 512 + b : 513 + b]}
                )
                nc.scalar.activation(
                    et[:],
                    sc[:],
                    mybir.ActivationFunctionType.Exp,
                    bias=cb_sb[:, 0:1],
                    scale=float(SCALE),
                    **accum,
                )
                # rel-bias correction factors: only block 0 deviates from 1
                # in the common curr_pos regime (full_fac covers the rest)
                if full_fac or b == 0:
                    etc = et_pool.tile([8, 512], mybir.dt.bfloat16, tag="etc")
                    nc.vector.tensor_mul(
                        etc[:], et[:], fac_sb[:, 512 * b : 512 * (b + 1)]
                    )
                    # S for corrected blocks: one DVE free-axis reduction
                    nc.vector.tensor_reduce(
                        out_sb[:, 512 + b : 513 + b],
                        etc[:],
                        axis=mybir.AxisListType.X,
                        op=mybir.AluOpType.add,
                    )
                else:
                    etc = et

                # transpose e^T -> e[l,h] in 4x [8,128] chunks (PE via identity)
                etp = etp_pool.tile([128, 4, 8], mybir.dt.bfloat16)
                for quad in range(4):
                    nc.tensor.transpose(
                        etp[:, quad],
                        etc[:, 128 * quad : 128 * (quad + 1)],
                        id_sb[:],
                    )
                e8 = e8_pool.tile([128, 4, 16], f8)
                nc.vector.tensor_copy(e8[:, :, 0:8], etp[:])

                for s in range(2):
                    nc.tensor.matmul(
                        u_ps[:],
                        e8[:, 2 * s : 2 * s + 2, 0:8],
                        zn_t[:, s],
                        start=(b == 0 and s == 0),
                        stop=(b == N_BLOCKS - 1 and s == 1),
                        perf_mode=DR,
                    )

            # final U copy on ACT (idle at kernel end, sits closest to PSUM)
            nc.scalar.copy(out_sb[:, 0:512], u_ps[:])
            nc.sync.dma_start(OUT[:], out_sb[:])

    nc.compile()
    return nc


def _get_program(full_fac: bool):
    if full_fac not in _cached:
        _cached[full_fac] = _build_program(full_fac)
    return _cached[full_fac]


def kernel(curr_pos, z_curr, z_past, Wq, bq, Wk, bk, Wv, bv, Wo, bo, rel_bias):
    curr_pos = int(np.asarray(curr_pos))
    z_curr = np.asarray(z_curr, dtype=np.float32)
    z_past = np.asarray(z_past, dtype=np.float32)
    Wq = np.asarray(Wq, dtype=np.float32)
    bq = np.asarray(bq, dtype=np.float32)
    Wk = np.asarray(Wk, dtype=np.float32)
    bk = np.asarray(bk, dtype=np.float32)
    Wv = np.asarray(Wv, dtype=np.float32)
    bv = np.asarray(bv, dtype=np.float32)
    Wo = np.asarray(Wo, dtype=np.float32)
    bo = np.asarray(bo, dtype=np.float32)
    rel_bias = np.asarray(rel_bias, dtype=np.float32)

    # ---- host-side O(D^2) prep (f64) ----
    q = z_curr.reshape(-1).astype(np.float64) @ Wq.T.astype(np.float64) + bq
    A = np.zeros((D2, HEADS), np.float64)
    c = np.zeros(HEADS, np.float64)
    for h in range(HEADS):
        sl = slice(h * 2 * HD, (h + 1) * 2 * HD)
        A[:, h] = Wk[sl, :].T.astype(np.float64) @ q[sl]
        c[h] = bk[sl].astype(np.float64) @ q[sl]
    relflat = rel_bias.reshape(2 * REL_MAX + 1, D2).astype(np.float64)
    rb = np.stack(
        [
            relflat[:, h * 2 * HD : (h + 1) * 2 * HD] @ q[h * 2 * HD : (h + 1) * 2 * HD]
            for h in range(HEADS)
        ],
        axis=1,
    )  # [129, 8]
    idx = np.clip(
        curr_pos - L_TOTAL + np.arange(L_TOTAL) + REL_MAX, 0, 2 * REL_MAX
    ).astype(np.int64)

    z8 = np.clip(z_past.reshape(L_TOTAL, D2), -240.0, 240.0).astype(FP8)
    A8 = np.clip(A, -240.0, 240.0).astype(np.float32).astype(FP8)
    a_dr = np.zeros((128, 2, 2, 16), FP8)
    a_dr[:, :, :, 0:8] = A8.reshape(2, 2, 128, HEADS).transpose(2, 0, 1, 3)

    ident = np.eye(8, dtype=BF16)

    in_maps = []
    facs = []
    for core in range(N_CORES):
        zc = z8[core * L_SHARD : (core + 1) * L_SHARD]
        # zt[sup, p, blk, cpair, d, l] = zc[512*(2*sup+blk) + l, 256*cpair + 128d + p]
        zt = np.ascontiguousarray(
            zc.reshape(N_SUPER, BLK_PER_SUPER, 512, 2, 2, 128).transpose(
                0, 5, 1, 3, 4, 2
            )
        )
        # zn[sup, p, blk, s, d, f] = zc[512*(2*sup+blk) + 256s + 128d + p, f]
        zn = np.ascontiguousarray(
            zc.reshape(N_SUPER, BLK_PER_SUPER, 2, 2, 128, D2).transpose(
                0, 4, 1, 2, 3, 5
            )
        )
        idx_c = idx[core * L_SHARD : (core + 1) * L_SHARD]
        base = int(np.bincount(idx_c, minlength=2 * REL_MAX + 1).argmax())
        cb = ((c + rb[base]) * SCALE).astype(np.float32).reshape(HEADS, 1)
        fac = np.ascontiguousarray(
            np.exp((rb[idx_c] - rb[base]) * SCALE).T.astype(BF16)
        )
        facs.append(fac)
        in_maps.append(
            {
                "zt": zt,
                "zn": zn,
                "a_dr": a_dr,
                "fac": fac,
                "cb": cb,
                "ident": ident,
            }
        )

    # fast path: correction factors are 1.0 outside block 0 on every core
    full_fac = any(
        not np.all(f[:, 512:] == np.asarray(1.0, BF16)) for f in facs
    )
    if not full_fac:
        for m in in_maps:
            m["fac"] = np.ascontiguousarray(m["fac"][:, 0:512])
    nc = _get_program(full_fac)
    res = run_bass_kernel_spmd(
        nc, in_maps, list(range(N_CORES)), trace=TRACE, **TRACE_KW
    )
    if TRACE:
        kernel.last_result = res

    U = np.zeros((HEADS, D2), np.float64)
    S = np.zeros(HEADS, np.float64)
    for r in res.results:
        o = np.asarray(r["out"], dtype=np.float64)
        U += o[:, 0:512]
        S += o[:, 512:528].sum(axis=1)

    hvec = np.zeros(D2, np.float64)
    for h in range(HEADS):
        sl = slice(h * 2 * HD, (h + 1) * 2 * HD)
        hvec[sl] = Wv[sl, :].astype(np.float64) @ (U[h] / S[h]) + bv[sl]
    out = hvec @ Wo.T.astype(np.float64) + bo
    return out.reshape(DIM, 2).astype(np.float32)
